# revision 25
# baseline (speedup 1.0000x reference)
"""Trainium2 Bass kernel for nn_AdjacencyLayer (gnn_message_passing).

Computes sim[i,j] = 1 / ((1-p)*msd[i,j] + p*mker[i,j]) with unit diagonal,
where msd = (|x_i|^2 + |x_j|^2 - 2 x_i.x_j)/d and mker = (e_i.e_j)/d with
e = exp(1 - dc).

Strategy (row parallelism across 8 NeuronCores, no collectives):
  - Each core owns a (1024, 8192) slab of the 8192x8192 output.
  - Per 128x512 output tile the denominator is ONE PSUM accumulation of
    4 K=128 bf16 matmuls (x and e parts, features 256 split 2x128, the
    row-side operands pre-scaled by -2(1-p)/d and p/d).
  - The rank-2 squared-norm terms a*sq_i + a*sq_j and the reciprocal are
    ONE fused custom-DVE op (7 of 8 ALU slices):
      out = recip1((psum + s0_per_partition) + sqj_tensor)
    with a bitwise-NOT seeded single-Newton reciprocal (minimax
    constants, ~1.7e-3 max rel err). The sq_j row is replicated across
    partitions once via GpSimd partition_broadcast.
  - Column-chunked rhs tiles + chunk-outer loop keep the PE fed from the
    first ~1MB of input DMA; output stores are batched 2048 wide (Sync
    dma_start issue costs ~600ns each).
  - Host pre-computes the transposed bf16 operand layouts (features on
    partitions) and the scalar factors; fixes the diagonal at gather.
"""

import os

import numpy as np
import ml_dtypes

import concourse.mybir as mybir
import concourse.tile as tile
from concourse import bacc
from concourse.bass_utils import run_bass_kernel_spmd

B = 8192
D = 256
N_CORES = 8
ROWS = B // N_CORES          # 1024 rows per core
MB = ROWS // 128             # 8 row blocks of 128

BF16 = mybir.dt.bfloat16
F32 = mybir.dt.float32

# Tuned for the single-Newton reciprocal: y0 = NOT(x)*C_SEED;
# y1 = y0*(C_NR - x*y0). Minimax over the x*bitcast(~x) in [-4.5,-4]
# interval gives max rel err ~1.73e-3 (same constants as the 2-NR op —
# they are already the 1-NR equioscillation optimum).
C_SEED = -0.23549792
C_NR = 2.0017324

_FUSED_OP = None


def _get_fused_op():
    """ADD2_RECIP_NR1_ANT: out = recip1((in0 + s0) + in1) — one DVE pass
    doing both squared-norm bias adds plus a seeded single-Newton
    reciprocal (7 of 8 ALU slices). Registered into concourse.dve_ops.OPS
    at runtime with a self-computed uops sha."""
    global _FUSED_OP
    if _FUSED_OP is not None:
        return _FUSED_OP
    import numpy as _np

    import concourse.dve_ops as dve_ops
    from concourse.dve_spec import C0, C1, C2, AluOp, Bin, Spec, Src0, Src1, lower
    from concourse.dve_uop import DveOpSpec

    _xp = (Src0 + C0) + Src1
    _nx = Bin(AluOp.BITWISE_NOT, _xp, _xp)
    _y0 = _nx * C1
    _body = _y0 * (C2 - _xp * _y0)

    def _ref(in0, in1, c0, c1, c2):
        xp = ((in0 + c0) + in1).astype(_np.float32)
        nx = (~xp.view(_np.int32)).view(_np.float32)
        y0 = nx * _np.float32(c1)
        return y0 * (_np.float32(c2) - xp * y0)

    spec = Spec(body=_body, reference=_ref)
    name = "ADD2_RECIP_NR1_ANT"
    shas = {}
    for ver in ("v3", "v4"):
        opcode = dve_ops._SUB_OPCODE_FOR_NAME.get(
            name, dve_ops._CUSTOM_DVE_ROW_BASE + len(dve_ops.OPS))
        shas[ver] = DveOpSpec(
            name=name, opcode=opcode, uops=lower(spec, ver=ver),
            rd1_en=True).sha(ver)
    op = dve_ops.DveOp(name, spec, subdim=False, uops_sha=shas)
    if name not in dve_ops._SUB_OPCODE_FOR_NAME:
        dve_ops._SUB_OPCODE_FOR_NAME[name] = (
            dve_ops._CUSTOM_DVE_ROW_BASE + len(dve_ops.OPS))
        dve_ops.OPS.append(op)
        dve_ops.CUSTOM_DVE_SPECS[name] = op.spec
    _FUSED_OP = op
    return op

# Exposed for test harnesses: the BassKernelResults of the last run.
LAST_RESULTS = None

_COMPILED_NC = None


def _install_trace_shim():
    """Provide antenv.axon_hooks (absent in this image) so that
    run_bass_kernel_spmd(trace=True) can capture NTFF profiles through the
    axon sidechannel. Mirrors trn_agent_boot._ntff_profile_via_ctypes."""
    import contextlib
    import ctypes
    import sys
    import types

    try:
        from antenv.axon_hooks import get_axon_ntff_profile_hook  # noqa: F401
        return
    except ImportError:
        pass

    so_path = "/opt/axon/libaxon_pjrt.so"
    if not os.path.exists(so_path):
        return
    lib = ctypes.CDLL(so_path)
    if not hasattr(lib, "axon_start_nrt_profile"):
        return
    lib.axon_start_nrt_profile.argtypes = [
        ctypes.POINTER(ctypes.c_int64),
        ctypes.c_size_t,
    ]
    lib.axon_start_nrt_profile.restype = ctypes.c_int64
    lib.axon_stop_nrt_profile.argtypes = [ctypes.c_char_p]
    lib.axon_stop_nrt_profile.restype = ctypes.c_int64

    @contextlib.contextmanager
    def _hook(output_dir, device_ids):
        import jax

        jax.devices()
        if device_ids:
            ids = (ctypes.c_int64 * len(device_ids))(*device_ids)
            rc = lib.axon_start_nrt_profile(ids, len(device_ids))
        else:
            rc = lib.axon_start_nrt_profile(None, 0)
        if rc != 0:
            raise RuntimeError(f"axon_start_nrt_profile rc={rc}")
        try:
            yield
        finally:
            n = lib.axon_stop_nrt_profile(str(output_dir).encode())
            print(f"ntff profile: {n} file(s) written to {output_dir}")

    mod = types.ModuleType("antenv.axon_hooks")
    mod.get_axon_ntff_profile_hook = lambda: _hook
    mod.set_axon_ntff_profile_hook = lambda h: None
    sys.modules["antenv.axon_hooks"] = mod


def _build_nc():
    fused_op = _get_fused_op()
    nc = bacc.Bacc(None, target_bir_lowering=False)

    rx0 = nc.dram_tensor("rx0", [128, B], BF16, kind="ExternalInput")
    rx1 = nc.dram_tensor("rx1", [128, B], BF16, kind="ExternalInput")
    re0 = nc.dram_tensor("re0", [128, B], BF16, kind="ExternalInput")
    re1 = nc.dram_tensor("re1", [128, B], BF16, kind="ExternalInput")
    lx0 = nc.dram_tensor("lx0", [128, ROWS], BF16, kind="ExternalInput")
    lx1 = nc.dram_tensor("lx1", [128, ROWS], BF16, kind="ExternalInput")
    le0 = nc.dram_tensor("le0", [128, ROWS], BF16, kind="ExternalInput")
    le1 = nc.dram_tensor("le1", [128, ROWS], BF16, kind="ExternalInput")
    sqj = nc.dram_tensor("sqj", [1, B], F32, kind="ExternalInput")
    sqi = nc.dram_tensor("sqi", [128, MB], F32, kind="ExternalInput")
    out = nc.dram_tensor("out", [ROWS, B], F32, kind="ExternalOutput")

    with tile.TileContext(nc) as tc:
        with (
            tc.tile_pool(name="const", bufs=1) as cpool,
            tc.tile_pool(name="psum", bufs=8, space="PSUM") as ppool,
            tc.tile_pool(name="outp", bufs=6) as opool,
        ):
            # One SBUF tile PER column chunk (not slices of one big tile):
            # chunk loads of one tensor would otherwise be WAW-serialized by
            # the tile dependency tracker, making the input stream ~5x slower.
            NCH = 4
            CH = B // NCH          # 2048 columns per chunk
            NPC = CH // 512        # n-tiles per chunk
            t_lx0 = cpool.tile([128, ROWS], BF16, tag="lx0")
            t_lx1 = cpool.tile([128, ROWS], BF16, tag="lx1")
            t_le0 = cpool.tile([128, ROWS], BF16, tag="le0")
            t_le1 = cpool.tile([128, ROWS], BF16, tag="le1")
            t_sqi = cpool.tile([128, MB], F32, tag="sqi")
            t_sqj = cpool.tile([1, B], F32, tag="sqj")
            rx0c, rx1c, re0c, re1c, sqb = [], [], [], [], []
            for ci in range(NCH):
                rx0c.append(cpool.tile([128, CH], BF16, name=f"rx0c{ci}", tag=f"rx0c{ci}"))
                rx1c.append(cpool.tile([128, CH], BF16, name=f"rx1c{ci}", tag=f"rx1c{ci}"))
                re0c.append(cpool.tile([128, CH], BF16, name=f"re0c{ci}", tag=f"re0c{ci}"))
                re1c.append(cpool.tile([128, CH], BF16, name=f"re1c{ci}", tag=f"re1c{ci}"))
                sqb.append(cpool.tile([128, CH], F32, name=f"sqb{ci}", tag=f"sqb{ci}"))

            # sqj first: the partition_broadcasts (GpSimd) depend on it and
            # gate the first fused-DVE epilogue op.
            nc.sync.dma_start(out=t_sqj[:], in_=sqj[:])
            nc.sync.dma_start(out=t_sqi[:], in_=sqi[:])
            nc.sync.dma_start(out=t_lx0[:], in_=lx0[:])
            nc.sync.dma_start(out=rx0c[0][:], in_=rx0[:, 0:CH])
            nc.sync.dma_start(out=t_lx1[:], in_=lx1[:])
            nc.sync.dma_start(out=rx1c[0][:], in_=rx1[:, 0:CH])
            nc.sync.dma_start(out=t_le0[:], in_=le0[:])
            nc.sync.dma_start(out=re0c[0][:], in_=re0[:, 0:CH])
            nc.sync.dma_start(out=t_le1[:], in_=le1[:])
            nc.sync.dma_start(out=re1c[0][:], in_=re1[:, 0:CH])
            for ci in range(1, NCH):
                cs = slice(ci * CH, (ci + 1) * CH)
                nc.sync.dma_start(out=rx0c[ci][:], in_=rx0[:, cs])
                nc.sync.dma_start(out=rx1c[ci][:], in_=rx1[:, cs])
                nc.sync.dma_start(out=re0c[ci][:], in_=re0[:, cs])
                nc.sync.dma_start(out=re1c[ci][:], in_=re1[:, cs])
            # Replicate the a*|x_j|^2 row across all 128 partitions so the
            # fused DVE epilogue can read it as a normal [128, 512] operand.
            # Chunk 0 is broadcast in 512-wide pieces: the whole-chunk op
            # takes ~8.7us on GpSimd and would gate the first DVE epilogue.
            for jj in range(NPC):
                nc.gpsimd.partition_broadcast(
                    sqb[0][:, jj * 512:(jj + 1) * 512],
                    t_sqj[0:1, jj * 512:(jj + 1) * 512])
            for ci in range(1, NCH):
                nc.gpsimd.partition_broadcast(
                    sqb[ci][:, :], t_sqj[0:1, ci * CH:(ci + 1) * CH])

            # Chunk-outer: each loaded column chunk feeds all 8 row
            # blocks (~28us of PE work per ~2MB chunk set), so the PE only
            # ever waits for chunk 0.
            lhs_k = None
            for c in range(NCH):
                for m in range(MB):
                    ms = slice(m * 128, (m + 1) * 128)
                    if c == 0 and m == 1:
                        continue  # handled by the m==0 super-group below
                    if c == 0 and m == 0:
                        # First super-group (row blocks 0 AND 1, all 8 PSUM
                        # banks) emitted k-major: the k0 matmuls of all 8
                        # tiles only need the first input pieces, so PE
                        # streams ~7us of work while the k1..k3 operands are
                        # still in flight instead of stalling on each k.
                        lhs_k = [t_lx0, t_lx1, t_le0, t_le1]
                        rhs_k = [rx0c[0], rx1c[0], re0c[0], re1c[0]]
                        ots = [opool.tile([128, CH], F32, name="ot", tag="ot")
                               for _ in range(2)]
                        pts = [ppool.tile([128, 512], F32, name="pt", tag="pt")
                               for _ in range(2 * NPC)]
                        for k in range(4):
                            for m2 in range(2):
                                ms2 = slice(m2 * 128, (m2 + 1) * 128)
                                for j in range(NPC):
                                    nc.tensor.matmul(
                                        pts[m2 * NPC + j][:], lhs_k[k][:, ms2],
                                        rhs_k[k][:, j * 512:(j + 1) * 512],
                                        start=(k == 0), stop=(k == 3))
                        for m2 in range(2):
                            ms2 = slice(m2 * 128, (m2 + 1) * 128)
                            for j in range(NPC):
                                js = slice(j * 512, (j + 1) * 512)
                                nc.vector._custom_dve(
                                    fused_op,
                                    out=ots[m2][:, js], in0=pts[m2 * NPC + j][:],
                                    in1=sqb[0][:, js],
                                    s0=t_sqi[:, m2:m2 + 1], s1=C_SEED, imm2=C_NR)
                            nc.sync.dma_start(out=out[ms2, 0:CH], in_=ots[m2][:])
                        continue
                    # one [128, 2048] output buffer per chunk: batches four
                    # 512-wide stores into one DMA (Sync issue is ~600ns per
                    # dma_start; 128 stores would congest the queue engine)
                    ot = opool.tile([128, CH], F32, tag="ot")
                    for j in range(NPC):
                        js = slice(j * 512, (j + 1) * 512)
                        pt = ppool.tile([128, 512], F32, tag="pt")
                        nc.tensor.matmul(pt[:], t_lx0[:, ms], rx0c[c][:, js],
                                         start=True, stop=False)
                        nc.tensor.matmul(pt[:], t_lx1[:, ms], rx1c[c][:, js],
                                         start=False, stop=False)
                        nc.tensor.matmul(pt[:], t_le0[:, ms], re0c[c][:, js],
                                         start=False, stop=False)
                        nc.tensor.matmul(pt[:], t_le1[:, ms], re1c[c][:, js],
                                         start=False, stop=True)
                        # out = 1/((psum + a*sq_i) + a*sq_j), single DVE pass
                        nc.vector._custom_dve(
                            fused_op,
                            out=ot[:, js], in0=pt[:], in1=sqb[c][:, js],
                            s0=t_sqi[:, m:m + 1], s1=C_SEED, imm2=C_NR)
                        if c == NCH - 1 and m == MB - 1:
                            # tail: store each 512-slice as soon as its recip
                            # lands instead of waiting for the whole 2048
                            nc.sync.dma_start(
                                out=out[ms, c * CH + j * 512:c * CH + (j + 1) * 512],
                                in_=ot[:, js])
                    if not (c == NCH - 1 and m == MB - 1):
                        nc.sync.dma_start(out=out[ms, c * CH:(c + 1) * CH], in_=ot[:])

    nc.compile()
    return nc


def kernel(x: np.ndarray, dc: np.ndarray, dc_param: np.ndarray) -> np.ndarray:
    global _COMPILED_NC, LAST_RESULTS

    x = np.ascontiguousarray(x, dtype=np.float32)
    dc = np.ascontiguousarray(dc, dtype=np.float32)
    p = np.float32(dc_param.reshape(-1)[0])
    a = np.float32((1.0 - p) / D)
    pd = np.float32(p / D)

    e = np.exp(np.float32(1.0) - dc, dtype=np.float32)
    sq = np.einsum("ij,ij->i", x, x, dtype=np.float32)
    asq = (a * sq).astype(np.float32)

    xt = np.ascontiguousarray(x.T)          # (256, 8192) f32
    et = np.ascontiguousarray(e.T)
    bf = ml_dtypes.bfloat16
    rx0 = xt[:128].astype(bf)
    rx1 = xt[128:].astype(bf)
    re0 = et[:128].astype(bf)
    re1 = et[128:].astype(bf)
    lxt = (np.float32(-2.0) * a) * xt       # (256, 8192) scaled lhs, f32
    let = pd * et

    sqj = np.ascontiguousarray(asq.reshape(1, B))

    in_maps = []
    for c in range(N_CORES):
        rs = slice(c * ROWS, (c + 1) * ROWS)
        # sqi[r, m] = a*|x_row|^2 for row m*128+r of this core's slab
        sqi_c = np.ascontiguousarray(asq[rs].reshape(MB, 128).T)
        in_maps.append({
            "rx0": rx0, "rx1": rx1, "re0": re0, "re1": re1,
            "lx0": lxt[:128, rs].astype(bf),
            "lx1": lxt[128:, rs].astype(bf),
            "le0": let[:128, rs].astype(bf),
            "le1": let[128:, rs].astype(bf),
            "sqj": sqj,
            "sqi": sqi_c,
        })

    if _COMPILED_NC is None:
        _COMPILED_NC = _build_nc()
    nc = _COMPILED_NC

    trace = bool(int(os.environ.get("KERNEL_TRACE", "0")))
    if trace:
        _install_trace_shim()
    res = run_bass_kernel_spmd(
        nc, in_maps, core_ids=list(range(N_CORES)),
        trace=trace,
    )
    LAST_RESULTS = res

    full = np.concatenate([res.results[c]["out"] for c in range(N_CORES)], axis=0)
    np.fill_diagonal(full, np.float32(1.0))
    return full
